# revision 28
# baseline (speedup 1.0000x reference)
"""Trainium2 Bass kernel for nn_Atte_module_77721728188895.

Module math (B=4, Q=K=256, H=D=768, NUM_LEN=100):
    s = q[:,:,None,:] + k[:,None,:,:] + emb[length_mask]          # [B,Q,K,H]
    att1 = einsum('bqkh,h->bqk', s, W1) + b1
    scores = softmax(einsum('bqkh,ch->bqkc', s, W2) + b2, -1)     # [B,Q,K,5]
    probs = softmax(att1 * attention_mask, -1)
    content = einsum('bqk,bkd->bqd', probs, m)                    # [B,Q,D]

Key identity: s only ever appears contracted against W1/W2 over H, so
    einsum('bqkh,ch->bqkc', s, W) = qW[b,q,c] + kW[b,k,c] + (emb@W.T)[lm[b,q,k],c]
The kernel never materializes s. Per core it computes the tiny projections
on the tensor engine, spills the 100x6 projected embedding table to DRAM,
and gathers it per (q,k) element with an indirect DMA using length_mask as
row offsets. Rank-1 broadcast terms are added via k=1 matmuls into PSUM.

Sharding: 8 cores, core i handles batch b=i//2, q-half i%2 (128 q rows).
"""

import os
import sys

for _p in ("/opt/trn_rl_repo", "/root/.axon_site/_ro/trn_rl_repo"):
    if os.path.isdir(_p) and _p not in sys.path:
        sys.path.insert(0, _p)

import numpy as np

import concourse.bass as bass
import concourse.bacc as bacc
import concourse.tile as tile
from concourse import mybir
from concourse.masks import make_identity
from concourse.tile_rust import add_dep_helper
from concourse import library_config

B, Q, K, H, D = 4, 256, 256, 768, 768
NL = 100          # length-embedding rows
C = 6             # gathered channels: [W1-proj, W2-proj x5]
P = 128           # q rows per core
NCORES = 8
EPAD = 64         # gather element size in f32 (256B DMA granularity)
NCHUNK = 32       # dma_gather limit: ~1024 idxs (desc ring capacity)
GCH = P * K // NCHUNK  # indices per gather chunk
F32 = mybir.dt.float32
I32 = mybir.dt.int32
I16 = mybir.dt.int16
AX = mybir.AxisListType
OP = mybir.AluOpType
ACT = mybir.ActivationFunctionType


def _bcast_ap(ap, extra):
    """Append a stride-0 dim of size `extra` to an AP."""
    return bass.AP(tensor=ap.tensor, offset=ap.offset, ap=list(ap.ap) + [[0, extra]])


def build_kernel():
    nc = bacc.Bacc("TRN2", target_bir_lowering=False, debug=False)

    # ---- DRAM I/O (per-core slices; SPMD: same program, different data) ----
    qs = nc.dram_tensor("qs", [P, H], F32, kind="ExternalInput")
    ks = nc.dram_tensor("ks", [K, H], F32, kind="ExternalInput")
    ms = nc.dram_tensor("ms", [K, D], F32, kind="ExternalInput")
    emb = nc.dram_tensor("emb", [NL, H], F32, kind="ExternalInput")
    w12 = nc.dram_tensor("w12", [C, H], F32, kind="ExternalInput")
    b12 = nc.dram_tensor("b12", [1, C], F32, kind="ExternalInput")
    am = nc.dram_tensor("am", [P, K], F32, kind="ExternalInput")
    # length_mask as an int16 index list in dma_gather's wrapped layout:
    # k-major order (e = k*128+q), split into NCHUNK gather chunks, each
    # wrapped over 16 partitions and replicated across the 8 Q7 cores.
    lmw = nc.dram_tensor("lmw", [128, NCHUNK, GCH // 16], I16, kind="ExternalInput")
    content = nc.dram_tensor("content", [P, D], F32, kind="ExternalOutput")
    scores = nc.dram_tensor("scores", [P, K * 5], F32, kind="ExternalOutput")
    # projected emb table scratch, rows padded to the 256B gather granularity
    t6 = nc.dram_tensor("t6", [128, EPAD], F32)

    with tile.TileContext(nc) as tc:
        _body(tc, qs, ks, ms, emb, w12, b12, am, lmw, content, scores, t6)

    nc.compile()
    return nc


def _body(tc, qs, ks, ms, emb, w12, b12, am, lmw, content, scores, t6, dbg=None):
    nc = tc.nc
    with (
        tc.tile_pool(name="const", bufs=1) as const,
        tc.tile_pool(name="io", bufs=1) as io,
        tc.tile_pool(name="work", bufs=1) as work,
        tc.tile_pool(name="tp_ps", bufs=2, space="PSUM") as tp_ps,
        tc.tile_pool(name="proj_ps", bufs=2, space="PSUM") as proj_ps,
        tc.tile_pool(name="big_ps", bufs=1, space="PSUM") as big_ps,
    ):
        # ---------------- constants ----------------
        ident = const.tile([128, 128], F32)
        make_identity(nc, ident[:])
        ones_row = const.tile([1, K], F32)
        nc.vector.memset(ones_row[:], 1.0)
        # one-hot selector masks: emask_q[j, c*128+q] = (j==c); emask_k likewise
        emask_q = const.tile([C, C, 128], F32)
        nc.gpsimd.memset(emask_q[:], 0.0)
        nc.gpsimd.affine_select(
            out=emask_q[:], in_=emask_q[:], compare_op=OP.not_equal, fill=1.0,
            base=0, pattern=[[1, C], [0, 128]], channel_multiplier=-1,
        )
        emask_k = const.tile([C, C, K], F32)
        nc.gpsimd.memset(emask_k[:], 0.0)
        nc.gpsimd.affine_select(
            out=emask_k[:], in_=emask_k[:], compare_op=OP.not_equal, fill=1.0,
            base=0, pattern=[[1, C], [0, K]], channel_multiplier=-1,
        )

        # ---------------- input DMAs (emb->table path first) ----------------
        emb_sb = io.tile([NL, H], F32)
        nc.sync.dma_start(emb_sb[:], emb[:])
        w12_sb = io.tile([C, H], F32)
        nc.sync.dma_start(w12_sb[:], w12[:])
        b12_sb = io.tile([1, C], F32)
        nc.sync.dma_start(b12_sb[:], b12[:])
        lmw_sb = io.tile([128, NCHUNK, GCH // 16], I16)
        nc.sync.dma_start(lmw_sb[:], lmw[:])
        am_sb = io.tile([P, K], F32)
        nc.sync.dma_start(am_sb[:], am[:])
        q_sb = io.tile([P, H], F32)
        nc.sync.dma_start(q_sb[:], qs[:])
        k_sb = []
        m_sb = []
        for r in range(2):
            kt = io.tile([128, H], F32, tag=f"k{r}")
            nc.sync.dma_start(kt[:], ks[r * 128:(r + 1) * 128, :])
            k_sb.append(kt)
            mt = io.tile([128, D], F32, tag=f"m{r}")
            nc.sync.dma_start(mt[:], ms[r * 128:(r + 1) * 128, :])
            m_sb.append(mt)

        NH = H // 128  # 6 contraction chunks

        # ---------------- W12^T: [6,768] -> 6x [128,6] ----------------
        w12t_ps = tp_ps.tile([128, 128], F32, tag="tp")
        for j in range(NH):
            nc.tensor.transpose(
                w12t_ps[:, j * C:(j + 1) * C],
                w12_sb[:, j * 128:(j + 1) * 128],
                ident[0:C, 0:C],
            )
        w12t_sb = const.tile([128, NH, C], F32)
        nc.vector.tensor_copy(w12t_sb[:], w12t_ps[:, 0:NH * C])

        # ---------------- emb^T and table projection ----------------
        embt_sb = work.tile([128, NH, NL], F32)
        for j in range(NH):
            tps = tp_ps.tile([128, 128], F32, tag="tp")
            nc.tensor.transpose(
                tps[:, 0:NL], emb_sb[:, j * 128:(j + 1) * 128], ident[0:NL, 0:NL]
            )
            nc.any.tensor_copy(embt_sb[:, j, :], tps[:, 0:NL])
        ew_ps = proj_ps.tile([NL, C], F32, tag="proj")
        for j in range(NH):
            nc.tensor.matmul(
                ew_ps[:], lhsT=embt_sb[:, j, :], rhs=w12t_sb[:, j, :],
                start=(j == 0), stop=False,
            )
        # fold biases into the table rows: += ones[100] x b12[1,6]
        nc.tensor.matmul(
            ew_ps[:], lhsT=ones_row[0:1, 0:NL], rhs=b12_sb[:],
            start=False, stop=True,
        )
        ew_sb = work.tile([128, EPAD], F32)
        nc.vector.memset(ew_sb[:], 0.0)
        nc.vector.tensor_copy(ew_sb[0:NL, 0:C], ew_ps[:])
        t6w = nc.sync.dma_start(t6[:], ew_sb[:])

        # ---------------- gather: g64[q, k, 0:6] = t6[lm[q,k], 0:6] --------
        # dma_gather places gathered row i at partition i%128, slot i//128;
        # with k-major index order (i = k*128+q) that is exactly [q, k, :].
        g64 = work.tile([P, K, EPAD], F32)
        libload = nc.gpsimd.load_library(library_config.mlp)
        for kc in range(NCHUNK):
            ksl = slice(kc * (K // NCHUNK), (kc + 1) * (K // NCHUNK))
            gi = nc.gpsimd.dma_gather(
                out_ap=g64[:, ksl, :],
                in_ap=t6[:],
                idxs_ap=lmw_sb[:, kc, :],
                num_idxs=GCH,
                num_idxs_reg=GCH,
                elem_size=EPAD,
            )
            add_dep_helper(gi.ins, t6w.ins, reason="table write -> gather")
            add_dep_helper(gi.ins, libload.ins, reason="mlp lib -> gather")

        # ---------------- q^T, k^T and projections ----------------
        qt_sb = work.tile([128, NH, 128], F32)
        for j in range(NH):
            tps = tp_ps.tile([128, 128], F32, tag="tp")
            nc.tensor.transpose(tps[:], q_sb[:, j * 128:(j + 1) * 128], ident[:])
            nc.any.tensor_copy(qt_sb[:, j, :], tps[:])
        kt_sb = work.tile([128, NH, K], F32)
        for j in range(NH):
            for r in range(2):
                tps = tp_ps.tile([128, 128], F32, tag="tp")
                nc.tensor.transpose(tps[:], k_sb[r][:, j * 128:(j + 1) * 128], ident[:])
                nc.any.tensor_copy(kt_sb[:, j, r * 128:(r + 1) * 128], tps[:])

        qw_ps = proj_ps.tile([P, C], F32, tag="proj")
        for j in range(NH):
            nc.tensor.matmul(
                qw_ps[:], lhsT=qt_sb[:, j, :], rhs=w12t_sb[:, j, :],
                start=(j == 0), stop=(j == NH - 1),
            )
        qw_sb = work.tile([P, C], F32)
        nc.vector.tensor_copy(qw_sb[:], qw_ps[:])

        kw_sb = []
        for r in range(2):
            kw_ps = proj_ps.tile([P, C], F32, tag="proj")
            for j in range(NH):
                nc.tensor.matmul(
                    kw_ps[:], lhsT=kt_sb[:, j, r * 128:(r + 1) * 128],
                    rhs=w12t_sb[:, j, :],
                    start=(j == 0), stop=(j == NH - 1),
                )
            kws = work.tile([P, C], F32, tag=f"kw{r}")
            nc.vector.tensor_copy(kws[:], kw_ps[:])
            kw_sb.append(kws)

        # transpose qw/kw to single-partition rows for broadcast matmuls
        qwt_ps = tp_ps.tile([128, 128], F32, tag="tp")
        nc.tensor.transpose(qwt_ps[0:C, :], qw_sb[:], ident[:])
        qwt_sb = work.tile([C, 128], F32)
        nc.vector.tensor_copy(qwt_sb[:], qwt_ps[0:C, :])
        kwt_sb = work.tile([C, K], F32)
        for r in range(2):
            kwt_ps = tp_ps.tile([128, 128], F32, tag="tp")
            nc.tensor.transpose(kwt_ps[0:C, :], kw_sb[r][:], ident[:])
            nc.vector.tensor_copy(kwt_sb[:, r * 128:(r + 1) * 128], kwt_ps[0:C, :])

        # ---------------- rank-1 logit terms into PSUM ----------------
        # rank1[q, c, k] = qw[q,c] + kw[k,c]
        rank1_ps = big_ps.tile([P, C, K], F32, tag="big")
        for c in range(C):
            # qw[q,c] broadcast over k: qwt.T @ onehot_c  (contraction over 6)
            nc.tensor.matmul(
                rank1_ps[:, c, :], lhsT=qwt_sb[:], rhs=emask_k[:, c, :],
                start=True, stop=False,
            )
            # kw[k,c] broadcast over q: onehot_c.T @ kwt
            nc.tensor.matmul(
                rank1_ps[:, c, :], lhsT=emask_q[:, c, :], rhs=kwt_sb[:],
                start=False, stop=True,
            )

        if dbg is not None:
            dbg("emask_q", emask_q[:].rearrange("a b c -> a (b c)"), (C, C * 128))
            dbg("emask_k", emask_k[:].rearrange("a b c -> a (b c)"), (C, C * K))
            dbg("qw", qw_sb[:], (P, C))
            dbg("kw0", kw_sb[0][:], (P, C))
            dbg("kw1", kw_sb[1][:], (P, C))
            dbg("ew", ew_sb[:, 0:C], (NL, C))
            dbg("g", g64[:, :, 0:C].rearrange("a b c -> a (b c)"), (P, K * C))
            r1dump = work.tile([P, C * K], F32)
            nc.vector.tensor_copy(r1dump[:], rank1_ps[:].rearrange("a b c -> a (b c)"))
            dbg("rank1", r1dump[:], (P, C * K))

        # ---------------- logits = gather + rank1 ----------------
        l_sb = work.tile([P, K, C], F32)
        nc.vector.tensor_tensor(
            out=l_sb[:], in0=g64[:, :, 0:C],
            in1=rank1_ps[:].rearrange("p c k -> p k c"),
            op=OP.add,
        )

        # ---------------- softmaxes, outputs ----------------
        s1_sb = work.tile([P, K], F32)
        e1_sb = work.tile([P, K], F32)
        z1 = work.tile([P, 2], F32)
        e5_sb = work.tile([P, K, 5], F32)
        dn_sb = work.tile([P, K], F32)
        rn_sb = work.tile([P, K], F32)
        sc_sb = work.tile([P, K, 5], F32)
        e1t_sb = work.tile([128, K], F32)

        for r in range(2):
            sl = slice(r * 128, (r + 1) * 128)
            # class-softmax path: scores = softmax_c(l[:,:,1:6])
            nc.scalar.activation(e5_sb[:, sl, :], l_sb[:, sl, 1:6], ACT.Exp)
            nc.vector.tensor_reduce(
                out=dn_sb[:, sl], in_=e5_sb[:, sl, :], axis=AX.X, op=OP.add
            )
            nc.vector.reciprocal(out=rn_sb[:, sl], in_=dn_sb[:, sl])
            nc.vector.tensor_tensor(
                out=sc_sb[:, sl, :], in0=e5_sb[:, sl, :],
                in1=_bcast_ap(rn_sb[:, sl], 5), op=OP.mult,
            )
            nc.sync.dma_start(scores[:, r * 640:(r + 1) * 640], sc_sb[:, sl, :])

            # K-softmax path: probs = softmax_k(l[:,:,0] * am)
            nc.vector.tensor_tensor(
                out=s1_sb[:, sl], in0=l_sb[:, sl, 0], in1=am_sb[:, sl], op=OP.mult
            )
            nc.scalar.activation(
                e1_sb[:, sl], s1_sb[:, sl], ACT.Exp, accum_out=z1[:, r:r + 1]
            )
            tps = tp_ps.tile([128, 128], F32, tag="tp")
            nc.tensor.transpose(tps[:], e1_sb[:, sl], ident[:])
            nc.any.tensor_copy(e1t_sb[:, sl], tps[:])

        zs = work.tile([P, 1], F32)
        nc.vector.tensor_tensor(out=zs[:], in0=z1[:, 0:1], in1=z1[:, 1:2], op=OP.add)
        zr = work.tile([P, 1], F32)
        nc.vector.reciprocal(out=zr[:], in_=zs[:])

        cont_big = big_ps.tile([P, C, K], F32, tag="big")
        cont_ps = cont_big[:, 0:3, :].rearrange("p a b -> p (a b)")
        for r in range(2):
            for n0, n1 in ((0, 512), (512, D)):
                nc.tensor.matmul(
                    cont_ps[:, n0:n1],
                    lhsT=e1t_sb[:, r * 128:(r + 1) * 128],
                    rhs=m_sb[r][:, n0:n1],
                    start=(r == 0), stop=(r == 1),
                )
        cont_sb = work.tile([P, D], F32)
        nc.vector.tensor_scalar_mul(cont_sb[:], cont_ps[:], zr[:])
        nc.sync.dma_start(content[:], cont_sb[:])


_NC_CACHE = None


def _get_nc():
    global _NC_CACHE
    if _NC_CACHE is None:
        _NC_CACHE = build_kernel()
    return _NC_CACHE


def make_in_maps(q, k, m, attention_mask, emb, W1, b1, W2, b2, length_mask):
    q = np.ascontiguousarray(np.asarray(q, np.float32))
    k = np.ascontiguousarray(np.asarray(k, np.float32))
    m = np.ascontiguousarray(np.asarray(m, np.float32))
    am = np.ascontiguousarray(np.asarray(attention_mask, np.float32))
    emb = np.ascontiguousarray(np.asarray(emb, np.float32))
    lm = np.asarray(length_mask).astype(np.int16)
    w12 = np.ascontiguousarray(
        np.concatenate([np.asarray(W1, np.float32), np.asarray(W2, np.float32)], 0)
    )
    b12 = np.ascontiguousarray(
        np.concatenate(
            [np.asarray(b1, np.float32), np.asarray(b2, np.float32)]
        ).reshape(1, C)
    )
    in_maps = []
    for i in range(NCORES):
        b, qh = divmod(i, 2)
        sl = slice(qh * P, (qh + 1) * P)
        # k-major index list, chunked, wrapped over 16 partitions, replicated
        idx = lm[b, sl].T.ravel()                                  # [P*K]
        w = idx.reshape(NCHUNK, GCH // 16, 16).transpose(0, 2, 1)  # [NCHUNK,16,S]
        lmw = np.ascontiguousarray(
            np.tile(w, (1, 8, 1)).transpose(1, 0, 2)               # [128,NCHUNK,S]
        )
        in_maps.append({
            "qs": q[b, sl],
            "ks": k[b],
            "ms": m[b],
            "emb": emb,
            "w12": w12,
            "b12": b12,
            "am": am[b, sl],
            "lmw": lmw,
        })
    return in_maps


def assemble_outputs(results):
    content = np.empty((B, Q, D), np.float32)
    scores = np.empty((B, Q, K, 5), np.float32)
    for i in range(NCORES):
        b, qh = divmod(i, 2)
        sl = slice(qh * P, (qh + 1) * P)
        content[b, sl] = results[i]["content"]
        scores[b, sl] = results[i]["scores"].reshape(P, K, 5)
    return content, scores


def kernel(**inputs):
    from concourse.bass_utils import run_bass_kernel_spmd

    nc = _get_nc()
    in_maps = make_in_maps(**inputs)
    res = run_bass_kernel_spmd(nc, in_maps, list(range(NCORES)))
    return assemble_outputs(res.results)


# revision 33
# speedup vs baseline: 1.6658x; 1.6658x over previous
"""Trainium2 Bass kernel for nn_Atte_module_77721728188895.

Module math (B=4, Q=K=256, H=D=768, NUM_LEN=100):
    s = q[:,:,None,:] + k[:,None,:,:] + emb[length_mask]          # [B,Q,K,H]
    att1 = einsum('bqkh,h->bqk', s, W1) + b1
    scores = softmax(einsum('bqkh,ch->bqkc', s, W2) + b2, -1)     # [B,Q,K,5]
    probs = softmax(att1 * attention_mask, -1)
    content = einsum('bqk,bkd->bqd', probs, m)                    # [B,Q,D]

Key identity: s only ever appears contracted against W1/W2 over H, so
    einsum('bqkh,ch->bqkc', s, W) = qW[b,q,c] + kW[b,k,c] + (emb@W.T)[lm[b,q,k],c]
The kernel never materializes s. Per core it computes the tiny projections
on the tensor engine, spills the 100x6 projected embedding table to DRAM,
and gathers it per (q,k) element with an indirect DMA using length_mask as
row offsets. Rank-1 broadcast terms are added via k=1 matmuls into PSUM.

Sharding: 8 cores, core i handles batch b=i//2, q-half i%2 (128 q rows).
"""

import os
import sys

for _p in ("/opt/trn_rl_repo", "/root/.axon_site/_ro/trn_rl_repo"):
    if os.path.isdir(_p) and _p not in sys.path:
        sys.path.insert(0, _p)

import numpy as np

import concourse.bass as bass
import concourse.bacc as bacc
import concourse.tile as tile
from concourse import mybir
from concourse.masks import make_identity
from concourse.tile_rust import add_dep_helper
from concourse import library_config

B, Q, K, H, D = 4, 256, 256, 768, 768
NL = 100          # length-embedding rows
C = 6             # gathered channels: [W1-proj, W2-proj x5]
P = 128           # q rows per core
NCORES = 8
NIC = 1024        # indices per indirect_copy (ISA dst elem-count limit)
NCALL = (P * K) // (8 * NIC)  # 4 calls; each call serves all 8 Q7 cores
F32 = mybir.dt.float32
I32 = mybir.dt.int32
I16 = mybir.dt.int16
U16 = mybir.dt.uint16
AX = mybir.AxisListType
OP = mybir.AluOpType
ACT = mybir.ActivationFunctionType


def _bcast_ap(ap, extra):
    """Append a stride-0 dim of size `extra` to an AP."""
    return bass.AP(tensor=ap.tensor, offset=ap.offset, ap=list(ap.ap) + [[0, extra]])


def build_kernel():
    nc = bacc.Bacc("TRN2", target_bir_lowering=False, debug=False)

    # ---- DRAM I/O (per-core slices; SPMD: same program, different data) ----
    qs = nc.dram_tensor("qs", [P, H], F32, kind="ExternalInput")
    ks = nc.dram_tensor("ks", [K, H], F32, kind="ExternalInput")
    ms = nc.dram_tensor("ms", [K, D], F32, kind="ExternalInput")
    emb = nc.dram_tensor("emb", [NL, H], F32, kind="ExternalInput")
    w12 = nc.dram_tensor("w12", [C, H], F32, kind="ExternalInput")
    b12 = nc.dram_tensor("b12", [1, C], F32, kind="ExternalInput")
    am = nc.dram_tensor("am", [P, K], F32, kind="ExternalInput")
    # length_mask index lists for gpsimd indirect_copy: Q7 core j gathers the
    # elements of q rows [16j,16j+16) in row-major order, NIC at a time, each
    # call's indices wrapped over the core's 16 partitions.
    lmw = nc.dram_tensor("lmw", [128, NCALL, NIC // 16], U16, kind="ExternalInput")
    content = nc.dram_tensor("content", [P, D], F32, kind="ExternalOutput")
    scores = nc.dram_tensor("scores", [P, K * 5], F32, kind="ExternalOutput")

    with tile.TileContext(nc) as tc:
        _body(tc, qs, ks, ms, emb, w12, b12, am, lmw, content, scores)

    nc.compile()
    return nc


def _body(tc, qs, ks, ms, emb, w12, b12, am, lmw, content, scores, dbg=None):
    nc = tc.nc
    with (
        tc.tile_pool(name="const", bufs=1) as const,
        tc.tile_pool(name="io", bufs=1) as io,
        tc.tile_pool(name="work", bufs=1) as work,
        tc.tile_pool(name="tp_ps", bufs=2, space="PSUM") as tp_ps,
        tc.tile_pool(name="proj_ps", bufs=2, space="PSUM") as proj_ps,
        tc.tile_pool(name="big_ps", bufs=1, space="PSUM") as big_ps,
    ):
        # ---------------- constants ----------------
        ident = const.tile([128, 128], F32)
        make_identity(nc, ident[:])
        ones_row = const.tile([1, K], F32)
        nc.vector.memset(ones_row[:], 1.0)
        # one-hot selector masks: emask_q[j, c*128+q] = (j==c); emask_k likewise
        emask_q = const.tile([C, C, 128], F32)
        nc.gpsimd.memset(emask_q[:], 0.0)
        nc.gpsimd.affine_select(
            out=emask_q[:], in_=emask_q[:], compare_op=OP.not_equal, fill=1.0,
            base=0, pattern=[[1, C], [0, 128]], channel_multiplier=-1,
        )
        emask_k = const.tile([C, C, K], F32)
        nc.gpsimd.memset(emask_k[:], 0.0)
        nc.gpsimd.affine_select(
            out=emask_k[:], in_=emask_k[:], compare_op=OP.not_equal, fill=1.0,
            base=0, pattern=[[1, C], [0, K]], channel_multiplier=-1,
        )

        # ---------------- input DMAs (emb->table path first) ----------------
        emb_sb = io.tile([NL, H], F32)
        nc.sync.dma_start(emb_sb[:], emb[:])
        w12_sb = io.tile([C, H], F32)
        nc.sync.dma_start(w12_sb[:], w12[:])
        b12_sb = io.tile([1, C], F32)
        nc.sync.dma_start(b12_sb[:], b12[:])
        lmw_sb = io.tile([128, NCALL, NIC // 16], U16)
        nc.sync.dma_start(lmw_sb[:], lmw[:])
        am_sb = io.tile([P, K], F32)
        nc.sync.dma_start(am_sb[:], am[:])
        q_sb = io.tile([P, H], F32)
        nc.sync.dma_start(q_sb[:], qs[:])
        k_sb = []
        m_sb = []
        for r in range(2):
            kt = io.tile([128, H], F32, tag=f"k{r}")
            nc.sync.dma_start(kt[:], ks[r * 128:(r + 1) * 128, :])
            k_sb.append(kt)
            mt = io.tile([128, D], F32, tag=f"m{r}")
            nc.sync.dma_start(mt[:], ms[r * 128:(r + 1) * 128, :])
            m_sb.append(mt)

        NH = H // 128  # 6 contraction chunks

        # ---------------- W12^T: [6,768] -> 6x [128,6] ----------------
        w12t_ps = tp_ps.tile([128, 128], F32, tag="tp")
        for j in range(NH):
            nc.tensor.transpose(
                w12t_ps[:, j * C:(j + 1) * C],
                w12_sb[:, j * 128:(j + 1) * 128],
                ident[0:C, 0:C],
            )
        w12t_sb = const.tile([128, NH, C], F32)
        nc.vector.tensor_copy(w12t_sb[:], w12t_ps[:, 0:NH * C])

        # ---------------- emb^T and table projection ----------------
        embt_sb = work.tile([128, NH, NL], F32)
        for j in range(NH):
            tps = tp_ps.tile([128, 128], F32, tag="tp")
            nc.tensor.transpose(
                tps[:, 0:NL], emb_sb[:, j * 128:(j + 1) * 128], ident[0:NL, 0:NL]
            )
            nc.any.tensor_copy(embt_sb[:, j, :], tps[:, 0:NL])
        ew_ps = proj_ps.tile([NL, C], F32, tag="proj")
        for j in range(NH):
            nc.tensor.matmul(
                ew_ps[:], lhsT=embt_sb[:, j, :], rhs=w12t_sb[:, j, :],
                start=(j == 0), stop=False,
            )
        # fold biases into the table rows: += ones[100] x b12[1,6]
        nc.tensor.matmul(
            ew_ps[:], lhsT=ones_row[0:1, 0:NL], rhs=b12_sb[:],
            start=False, stop=True,
        )
        ew_sb = work.tile([NL, C], F32)
        nc.vector.tensor_copy(ew_sb[:], ew_ps[:])
        # transpose to [6, 100] rows, then replicate to every 16-partition
        # group so each Q7 core sees the 6 channel rows of the table.
        ewt_ps = tp_ps.tile([128, 128], F32, tag="tp")
        nc.tensor.transpose(ewt_ps[0:C, 0:NL], ew_sb[:], ident[0:NL, 0:NL])
        ewt_sb = work.tile([C, NL], F32)
        nc.vector.tensor_copy(ewt_sb[:], ewt_ps[0:C, 0:NL])
        t6_sb = work.tile([128, NL], F32)
        nc.vector.memset(t6_sb[:], 0.0)
        twr = []
        for j in range(8):
            wj = nc.sync.dma_start(t6_sb[16 * j:16 * j + C, :], ewt_sb[:])
            twr.append(wj)

        # ---------------- gather: go[16j+t, i] = ewT[t, lm[16j+i//256, i%256]]
        go = work.tile([128, NCALL * NIC], F32)
        gis = []
        for c in range(NCALL):
            gi = nc.gpsimd.indirect_copy(
                out=go[:, c * NIC:(c + 1) * NIC],
                data=t6_sb[:],
                idxs=lmw_sb[:, c, :],
                i_know_ap_gather_is_preferred=True,
            )
            for wj in twr:
                add_dep_helper(gi.ins, wj.ins, reason="table -> gather")
            gis.append(gi)

        # redistribute to channel planes: gpl[q, c, k] = go[16*(q//16)+c, ...]
        gpl = work.tile([P, C, K], F32)
        for c in range(C):
            src = go[:].rearrange("p (u k) -> p u k", u=16)[c:c + 113:16, :, :]
            rd = nc.sync.dma_start(gpl[:, c, :], src)
            for gi in gis:
                add_dep_helper(rd.ins, gi.ins, reason="gather -> redistribute")

        # ---------------- q^T, k^T and projections ----------------
        qt_sb = work.tile([128, NH, 128], F32)
        for j in range(NH):
            tps = tp_ps.tile([128, 128], F32, tag="tp")
            nc.tensor.transpose(tps[:], q_sb[:, j * 128:(j + 1) * 128], ident[:])
            nc.any.tensor_copy(qt_sb[:, j, :], tps[:])
        kt_sb = work.tile([128, NH, K], F32)
        for j in range(NH):
            for r in range(2):
                tps = tp_ps.tile([128, 128], F32, tag="tp")
                nc.tensor.transpose(tps[:], k_sb[r][:, j * 128:(j + 1) * 128], ident[:])
                nc.any.tensor_copy(kt_sb[:, j, r * 128:(r + 1) * 128], tps[:])

        qw_ps = proj_ps.tile([P, C], F32, tag="proj")
        for j in range(NH):
            nc.tensor.matmul(
                qw_ps[:], lhsT=qt_sb[:, j, :], rhs=w12t_sb[:, j, :],
                start=(j == 0), stop=(j == NH - 1),
            )
        qw_sb = work.tile([P, C], F32)
        nc.vector.tensor_copy(qw_sb[:], qw_ps[:])

        kw_sb = []
        for r in range(2):
            kw_ps = proj_ps.tile([P, C], F32, tag="proj")
            for j in range(NH):
                nc.tensor.matmul(
                    kw_ps[:], lhsT=kt_sb[:, j, r * 128:(r + 1) * 128],
                    rhs=w12t_sb[:, j, :],
                    start=(j == 0), stop=(j == NH - 1),
                )
            kws = work.tile([P, C], F32, tag=f"kw{r}")
            nc.vector.tensor_copy(kws[:], kw_ps[:])
            kw_sb.append(kws)

        # transpose qw/kw to single-partition rows for broadcast matmuls
        qwt_ps = tp_ps.tile([128, 128], F32, tag="tp")
        nc.tensor.transpose(qwt_ps[0:C, :], qw_sb[:], ident[:])
        qwt_sb = work.tile([C, 128], F32)
        nc.vector.tensor_copy(qwt_sb[:], qwt_ps[0:C, :])
        kwt_sb = work.tile([C, K], F32)
        for r in range(2):
            kwt_ps = tp_ps.tile([128, 128], F32, tag="tp")
            nc.tensor.transpose(kwt_ps[0:C, :], kw_sb[r][:], ident[:])
            nc.vector.tensor_copy(kwt_sb[:, r * 128:(r + 1) * 128], kwt_ps[0:C, :])

        # ---------------- rank-1 logit terms into PSUM ----------------
        # rank1[q, c, k] = qw[q,c] + kw[k,c]
        rank1_ps = big_ps.tile([P, C, K], F32, tag="big")
        for c in range(C):
            # qw[q,c] broadcast over k: qwt.T @ onehot_c  (contraction over 6)
            nc.tensor.matmul(
                rank1_ps[:, c, :], lhsT=qwt_sb[:], rhs=emask_k[:, c, :],
                start=True, stop=False,
            )
            # kw[k,c] broadcast over q: onehot_c.T @ kwt
            nc.tensor.matmul(
                rank1_ps[:, c, :], lhsT=emask_q[:, c, :], rhs=kwt_sb[:],
                start=False, stop=True,
            )

        if dbg is not None:
            dbg("qw", qw_sb[:], (P, C))
            dbg("kw0", kw_sb[0][:], (P, C))
            dbg("kw1", kw_sb[1][:], (P, C))
            dbg("ew", ew_sb[:], (NL, C))
            dbg("g", gpl[:].rearrange("a b c -> a (b c)"), (P, C * K))
            r1dump = work.tile([P, C * K], F32)
            nc.vector.tensor_copy(r1dump[:], rank1_ps[:].rearrange("a b c -> a (b c)"))
            dbg("rank1", r1dump[:], (P, C * K))

        # ---------------- logits = gather + rank1 (channel planes) ----------
        l_sb = work.tile([P, C, K], F32)
        nc.vector.tensor_tensor(out=l_sb[:], in0=gpl[:], in1=rank1_ps[:], op=OP.add)

        # ---------------- softmaxes, outputs ----------------
        s1_sb = work.tile([P, K], F32)
        e1_sb = work.tile([P, K], F32)
        z1 = work.tile([P, 1], F32)
        e5_sb = work.tile([P, 5, K], F32)
        dn_sb = work.tile([P, K], F32)
        rn_sb = work.tile([P, K], F32)
        sc_sb = work.tile([P, K, 5], F32)
        e1t_sb = work.tile([128, K], F32)

        # class-softmax: scores[q,k,c] = softmax_c over the 5 planes
        nc.scalar.activation(e5_sb[:], l_sb[:, 1:6, :], ACT.Exp)
        nc.vector.tensor_reduce(
            out=dn_sb[:], in_=e5_sb[:].rearrange("p c k -> p k c"),
            axis=AX.X, op=OP.add,
        )
        nc.vector.reciprocal(out=rn_sb[:], in_=dn_sb[:])
        rn_b = bass.AP(
            tensor=rn_sb[:].tensor, offset=rn_sb[:].offset,
            ap=[rn_sb[:].ap[0], [0, 5], rn_sb[:].ap[1]],
        )
        nc.vector.tensor_tensor(
            out=sc_sb[:].rearrange("p k c -> p c k"), in0=e5_sb[:], in1=rn_b,
            op=OP.mult,
        )
        for r in range(2):
            sl = slice(r * 128, (r + 1) * 128)
            nc.sync.dma_start(scores[:, r * 640:(r + 1) * 640], sc_sb[:, sl, :])

        # K-softmax path: probs = softmax_k(l[:,0,:] * am)
        nc.vector.tensor_tensor(
            out=s1_sb[:], in0=l_sb[:, 0, :], in1=am_sb[:], op=OP.mult
        )
        nc.scalar.activation(e1_sb[:], s1_sb[:], ACT.Exp, accum_out=z1[:])
        for r in range(2):
            sl = slice(r * 128, (r + 1) * 128)
            tps = tp_ps.tile([128, 128], F32, tag="tp")
            nc.tensor.transpose(tps[:], e1_sb[:, sl], ident[:])
            nc.any.tensor_copy(e1t_sb[:, sl], tps[:])

        zr = work.tile([P, 1], F32)
        nc.vector.reciprocal(out=zr[:], in_=z1[:])
        cont_big = big_ps.tile([P, C, K], F32, tag="big")
        cont_ps = cont_big[:, 0:3, :].rearrange("p a b -> p (a b)")
        for r in range(2):
            for n0, n1 in ((0, 512), (512, D)):
                nc.tensor.matmul(
                    cont_ps[:, n0:n1],
                    lhsT=e1t_sb[:, r * 128:(r + 1) * 128],
                    rhs=m_sb[r][:, n0:n1],
                    start=(r == 0), stop=(r == 1),
                )
        cont_sb = work.tile([P, D], F32)
        nc.vector.tensor_scalar_mul(cont_sb[:], cont_ps[:], zr[:])
        nc.sync.dma_start(content[:], cont_sb[:])


_NC_CACHE = None


def _get_nc():
    global _NC_CACHE
    if _NC_CACHE is None:
        _NC_CACHE = build_kernel()
    return _NC_CACHE


def make_in_maps(q, k, m, attention_mask, emb, W1, b1, W2, b2, length_mask):
    q = np.ascontiguousarray(np.asarray(q, np.float32))
    k = np.ascontiguousarray(np.asarray(k, np.float32))
    m = np.ascontiguousarray(np.asarray(m, np.float32))
    am = np.ascontiguousarray(np.asarray(attention_mask, np.float32))
    emb = np.ascontiguousarray(np.asarray(emb, np.float32))
    lm = np.asarray(length_mask).astype(np.uint16)
    w12 = np.ascontiguousarray(
        np.concatenate([np.asarray(W1, np.float32), np.asarray(W2, np.float32)], 0)
    )
    b12 = np.ascontiguousarray(
        np.concatenate(
            [np.asarray(b1, np.float32), np.asarray(b2, np.float32)]
        ).reshape(1, C)
    )
    in_maps = []
    for i in range(NCORES):
        b, qh = divmod(i, 2)
        sl = slice(qh * P, (qh + 1) * P)
        # per-Q7-core row-major index lists, wrapped over 16 partitions
        lst = lm[b, sl].reshape(8, NCALL, NIC // 16, 16)           # [j,c,s,t]
        lmw = np.ascontiguousarray(
            lst.transpose(0, 3, 1, 2).reshape(128, NCALL, NIC // 16)
        )
        in_maps.append({
            "qs": q[b, sl],
            "ks": k[b],
            "ms": m[b],
            "emb": emb,
            "w12": w12,
            "b12": b12,
            "am": am[b, sl],
            "lmw": lmw,
        })
    return in_maps


def assemble_outputs(results):
    content = np.empty((B, Q, D), np.float32)
    scores = np.empty((B, Q, K, 5), np.float32)
    for i in range(NCORES):
        b, qh = divmod(i, 2)
        sl = slice(qh * P, (qh + 1) * P)
        content[b, sl] = results[i]["content"]
        scores[b, sl] = results[i]["scores"].reshape(P, K, 5)
    return content, scores


def kernel(**inputs):
    from concourse.bass_utils import run_bass_kernel_spmd

    nc = _get_nc()
    in_maps = make_in_maps(**inputs)
    res = run_bass_kernel_spmd(nc, in_maps, list(range(NCORES)))
    return assemble_outputs(res.results)


# revision 35
# speedup vs baseline: 3.1901x; 1.9151x over previous
"""Trainium2 Bass kernel for nn_Atte_module_77721728188895.

Module math (B=4, Q=K=256, H=D=768, NUM_LEN=100):
    s = q[:,:,None,:] + k[:,None,:,:] + emb[length_mask]          # [B,Q,K,H]
    att1 = einsum('bqkh,h->bqk', s, W1) + b1
    scores = softmax(einsum('bqkh,ch->bqkc', s, W2) + b2, -1)     # [B,Q,K,5]
    probs = softmax(att1 * attention_mask, -1)
    content = einsum('bqk,bkd->bqd', probs, m)                    # [B,Q,D]

Key identity: s only ever appears contracted against W1/W2 over H, so
    einsum('bqkh,ch->bqkc', s, W) = qW[b,q,c] + kW[b,k,c] + (emb@W.T)[lm[b,q,k],c]
The kernel never materializes s.

The table gather emb_proj[lm] is computed without gpsimd (whose indexed ops
cost ~27ns/index on trn2): length_mask is partition-broadcast as fp16, one
`is_equal` tensor_scalar against a per-partition iota builds an exact one-hot
[100, e] matrix at DVE 4x fp16 rate, and 128-element chunks of it are
contracted against the projected 100x6 table on the tensor engine, which
lands gathered rows directly in interleaved [q, k, c] layout (k-major
element order puts q on partitions).

Sharding: 8 cores, core i handles batch b=i//2, q-half i%2 (128 q rows).
"""

import os
import sys

for _p in ("/opt/trn_rl_repo", "/root/.axon_site/_ro/trn_rl_repo"):
    if os.path.isdir(_p) and _p not in sys.path:
        sys.path.insert(0, _p)

import numpy as np

import concourse.bass as bass
import concourse.bacc as bacc
import concourse.tile as tile
from concourse import mybir

B, Q, K, H, D = 4, 256, 256, 768, 768
NL = 100          # length-embedding rows
C = 6             # channels: [W1-proj, W2-proj x5]
P = 128           # q rows per core
NCORES = 8
NQT = 4           # k-quarters for the one-hot pipeline
KQ = K // NQT     # 64 k's per quarter -> 8192 elements
F32 = mybir.dt.float32
F16 = mybir.dt.float16
AX = mybir.AxisListType
OP = mybir.AluOpType
ACT = mybir.ActivationFunctionType


def _bcast_part(ap, n):
    """Broadcast a [1, ...] AP across n partitions (stride-0 partition dim)."""
    return bass.AP(tensor=ap.tensor, offset=ap.offset,
                   ap=[[0, n]] + list(ap.ap[1:]))


def build_kernel():
    nc = bacc.Bacc("TRN2", target_bir_lowering=False, debug=False)

    qs = nc.dram_tensor("qs", [P, H], F32, kind="ExternalInput")
    ks = nc.dram_tensor("ks", [K, H], F32, kind="ExternalInput")
    ms = nc.dram_tensor("ms", [K, D], F32, kind="ExternalInput")
    emb = nc.dram_tensor("emb", [NL, H], F32, kind="ExternalInput")
    w12 = nc.dram_tensor("w12", [C, H], F32, kind="ExternalInput")
    b12 = nc.dram_tensor("b12", [1, C], F32, kind="ExternalInput")
    am = nc.dram_tensor("am", [P, K], F32, kind="ExternalInput")
    # k-major flat length_mask as fp16 (values < 100 exact)
    lmf = nc.dram_tensor("lmf", [1, P * K], F16, kind="ExternalInput")
    # host-provided constants (keeps the Pool engine fully idle)
    ident = nc.dram_tensor("ident", [128, 128], F32, kind="ExternalInput")
    emq = nc.dram_tensor("emq", [C, C * 128], F32, kind="ExternalInput")
    emk = nc.dram_tensor("emk", [C, C * K], F32, kind="ExternalInput")
    iota = nc.dram_tensor("iota", [128, 1], F32, kind="ExternalInput")
    content = nc.dram_tensor("content", [P, D], F32, kind="ExternalOutput")
    scores = nc.dram_tensor("scores", [P, K * 5], F32, kind="ExternalOutput")

    with tile.TileContext(nc) as tc:
        _body(tc, qs, ks, ms, emb, w12, b12, am, lmf, ident, emq, emk, iota,
              content, scores)

    nc.compile()
    return nc


def _body(tc, qs, ks, ms, emb, w12, b12, am, lmf, ident, emq, emk, iota,
          content, scores, dbg=None):
    nc = tc.nc
    with (
        tc.tile_pool(name="const", bufs=1) as const,
        tc.tile_pool(name="io", bufs=1) as io,
        tc.tile_pool(name="work", bufs=1) as work,
        tc.tile_pool(name="oh16", bufs=2) as oh16,
        tc.tile_pool(name="tp_ps", bufs=2, space="PSUM") as tp_ps,
        tc.tile_pool(name="proj_ps", bufs=1, space="PSUM") as proj_ps,
        tc.tile_pool(name="big_ps", bufs=1, space="PSUM") as big_ps,
        tc.tile_pool(name="cons_ps", bufs=2, space="PSUM") as cons_ps,
    ):
        # ---------------- constants / inputs ----------------
        ident_sb = const.tile([128, 128], F32)
        nc.sync.dma_start(ident_sb[:], ident[:])
        emq_sb = const.tile([C, C, 128], F32)
        nc.sync.dma_start(emq_sb[:].rearrange("a b c -> a (b c)"), emq[:])
        emk_sb = const.tile([C, C, K], F32)
        nc.sync.dma_start(emk_sb[:].rearrange("a b c -> a (b c)"), emk[:])
        iota_sb = const.tile([128, 1], F32)
        nc.sync.dma_start(iota_sb[:], iota[:])
        ones_row = const.tile([1, K], F32)
        nc.vector.memset(ones_row[:], 1.0)

        # length-mask partition-broadcasts (quarter 0 first: critical path)
        lmb = []
        for h in range(NQT):
            t = oh16.tile([128, KQ * 128], F16, tag=f"lmb{h % 2}")
            nc.sync.dma_start(
                t[:],
                _bcast_part(lmf[0:1, h * KQ * 128:(h + 1) * KQ * 128], 128),
            )
            lmb.append(t)

        emb_sb = io.tile([NL, H], F32)
        nc.sync.dma_start(emb_sb[:], emb[:])
        w12_sb = io.tile([C, H], F32)
        nc.sync.dma_start(w12_sb[:], w12[:])
        b12_sb = io.tile([1, C], F32)
        nc.sync.dma_start(b12_sb[:], b12[:])
        am_sb = io.tile([P, K], F32)
        nc.sync.dma_start(am_sb[:], am[:])
        q_sb = io.tile([P, H], F32)
        nc.sync.dma_start(q_sb[:], qs[:])
        k_sb = []
        m_sb = []
        for r in range(2):
            kt = io.tile([128, H], F32, tag=f"k{r}")
            nc.sync.dma_start(kt[:], ks[r * 128:(r + 1) * 128, :])
            k_sb.append(kt)
            mt = io.tile([128, D], F32, tag=f"m{r}")
            nc.sync.dma_start(mt[:], ms[r * 128:(r + 1) * 128, :])
            m_sb.append(mt)

        NH = H // 128

        # ---------------- W12^T, emb^T, table projection ----------------
        w12t_ps = tp_ps.tile([128, 128], F32, tag="tp")
        for j in range(NH):
            nc.tensor.transpose(
                w12t_ps[:, j * C:(j + 1) * C],
                w12_sb[:, j * 128:(j + 1) * 128],
                ident_sb[0:C, 0:C],
            )
        w12t_sb = const.tile([128, NH, C], F32)
        nc.vector.tensor_copy(w12t_sb[:], w12t_ps[:, 0:NH * C])

        embt_sb = work.tile([128, NH, NL], F32)
        for j in range(NH):
            tps = tp_ps.tile([128, 128], F32, tag="tp")
            nc.tensor.transpose(
                tps[:, 0:NL], emb_sb[:, j * 128:(j + 1) * 128],
                ident_sb[0:NL, 0:NL],
            )
            nc.any.tensor_copy(embt_sb[:, j, :], tps[:, 0:NL])
        ew_ps = proj_ps.tile([NL, C], F32, tag="proj")
        for j in range(NH):
            nc.tensor.matmul(
                ew_ps[:], lhsT=embt_sb[:, j, :], rhs=w12t_sb[:, j, :],
                start=(j == 0), stop=False,
            )
        nc.tensor.matmul(  # fold biases: += ones[100] x b12[1,6]
            ew_ps[:], lhsT=ones_row[0:1, 0:NL], rhs=b12_sb[:],
            start=False, stop=True,
        )
        t6_sb = work.tile([NL, C], F16)
        nc.vector.tensor_copy(t6_sb[:], ew_ps[:])

        # ---------------- q^T, k^T and projections ----------------
        qt_sb = work.tile([128, NH, 128], F32)
        for j in range(NH):
            tps = tp_ps.tile([128, 128], F32, tag="tp")
            nc.tensor.transpose(tps[:], q_sb[:, j * 128:(j + 1) * 128],
                                ident_sb[:])
            nc.any.tensor_copy(qt_sb[:, j, :], tps[:])
        kt_sb = work.tile([128, NH, K], F32)
        for j in range(NH):
            for r in range(2):
                tps = tp_ps.tile([128, 128], F32, tag="tp")
                nc.tensor.transpose(tps[:], k_sb[r][:, j * 128:(j + 1) * 128],
                                    ident_sb[:])
                nc.any.tensor_copy(kt_sb[:, j, r * 128:(r + 1) * 128], tps[:])

        qw_ps = proj_ps.tile([P, C], F32, tag="proj")
        for j in range(NH):
            nc.tensor.matmul(
                qw_ps[:], lhsT=qt_sb[:, j, :], rhs=w12t_sb[:, j, :],
                start=(j == 0), stop=(j == NH - 1),
            )
        qw_sb = work.tile([P, C], F32)
        nc.vector.tensor_copy(qw_sb[:], qw_ps[:])

        kw_sb = []
        for r in range(2):
            kw_ps = proj_ps.tile([P, C], F32, tag="proj")
            for j in range(NH):
                nc.tensor.matmul(
                    kw_ps[:], lhsT=kt_sb[:, j, r * 128:(r + 1) * 128],
                    rhs=w12t_sb[:, j, :],
                    start=(j == 0), stop=(j == NH - 1),
                )
            kws = work.tile([P, C], F32, tag=f"kw{r}")
            nc.vector.tensor_copy(kws[:], kw_ps[:])
            kw_sb.append(kws)

        qwt_ps = tp_ps.tile([128, 128], F32, tag="tp")
        nc.tensor.transpose(qwt_ps[0:C, :], qw_sb[:], ident_sb[:])
        qwt_sb = work.tile([C, 128], F32)
        nc.vector.tensor_copy(qwt_sb[:], qwt_ps[0:C, :])
        kwt_sb = work.tile([C, K], F32)
        for r in range(2):
            kwt_ps = tp_ps.tile([128, 128], F32, tag="tp")
            nc.tensor.transpose(kwt_ps[0:C, :], kw_sb[r][:], ident_sb[:])
            nc.vector.tensor_copy(kwt_sb[:, r * 128:(r + 1) * 128],
                                  kwt_ps[0:C, :])

        # ---------------- rank-1 terms, copied to SBUF interleaved ----------
        rank1_ps = big_ps.tile([P, C, K], F32, tag="big")
        for c in range(C):
            nc.tensor.matmul(
                rank1_ps[:, c, :], lhsT=qwt_sb[:], rhs=emk_sb[:, c, :],
                start=True, stop=False,
            )
            nc.tensor.matmul(
                rank1_ps[:, c, :], lhsT=emq_sb[:, c, :], rhs=kwt_sb[:],
                start=False, stop=True,
            )
        rank1_sb = work.tile([P, K, C], F32)
        nc.vector.tensor_copy(
            rank1_sb[:], rank1_ps[:].rearrange("p c k -> p k c")
        )

        # ---------------- gather via one-hot + PE, fused epilogue ----------
        l_sb = work.tile([P, K, C], F32)
        s1_sb = work.tile([P, K], F32)
        e1_sb = work.tile([P, K], F32)
        z1 = work.tile([P, NQT], F32)
        e5_sb = work.tile([P, K, 5], F32)
        dn_sb = work.tile([P, K], F32)
        rn_sb = work.tile([P, K], F32)
        sc_sb = work.tile([P, K, 5], F32)
        e1t_sb = work.tile([128, K], F32)

        for h in range(NQT):
            sl = slice(h * KQ, (h + 1) * KQ)
            # one-hot: oh[l, e] = (lm[e] == l), exact in fp16
            oh = oh16.tile([128, KQ * 128], F16, tag=f"oh{h % 2}")
            nc.vector.tensor_scalar(
                out=oh[:], in0=lmb[h][:], scalar1=iota_sb[:], scalar2=None,
                op0=OP.is_equal,
            )
            # contract one-hot chunks against the table: out rows land as
            # [q, 6] for each k (k-major flattening puts q on partitions)
            cons = cons_ps.tile([128, KQ * C], F32, tag="cons")
            for kk in range(KQ):
                nc.tensor.matmul(
                    cons[:, kk * C:(kk + 1) * C],
                    lhsT=oh[0:NL, kk * 128:(kk + 1) * 128],
                    rhs=t6_sb[:],
                    start=True, stop=True,
                )
            # logits = gathered + rank1
            nc.vector.tensor_tensor(
                out=l_sb[:, sl, :],
                in0=cons[:].rearrange("p (k c) -> p k c", c=C),
                in1=rank1_sb[:, sl, :], op=OP.add,
            )
            # class softmax over the 5 W2 channels
            nc.scalar.activation(
                e5_sb[:, sl, :], l_sb[:, sl, 1:6], ACT.Exp
            )
            nc.vector.tensor_reduce(
                out=dn_sb[:, sl], in_=e5_sb[:, sl, :], axis=AX.X, op=OP.add
            )
            nc.vector.reciprocal(out=rn_sb[:, sl], in_=dn_sb[:, sl])
            rn_b = bass.AP(
                tensor=rn_sb[:].tensor, offset=rn_sb[:, sl].offset,
                ap=[rn_sb[:].ap[0], [1, KQ], [0, 5]],
            )
            nc.vector.tensor_tensor(
                out=sc_sb[:, sl, :], in0=e5_sb[:, sl, :], in1=rn_b,
                op=OP.mult,
            )
            nc.sync.dma_start(
                scores[:, h * KQ * 5:(h + 1) * KQ * 5], sc_sb[:, sl, :]
            )
            # K-softmax path
            nc.vector.tensor_tensor(
                out=s1_sb[:, sl], in0=l_sb[:, sl, 0], in1=am_sb[:, sl],
                op=OP.mult,
            )
            nc.scalar.activation(
                e1_sb[:, sl], s1_sb[:, sl], ACT.Exp,
                accum_out=z1[:, h:h + 1],
            )

        for r in range(2):
            sl = slice(r * 128, (r + 1) * 128)
            tps = tp_ps.tile([128, 128], F32, tag="tp")
            nc.tensor.transpose(tps[:], e1_sb[:, sl], ident_sb[:])
            nc.any.tensor_copy(e1t_sb[:, sl], tps[:])

        zs = work.tile([P, 1], F32)
        nc.vector.tensor_reduce(out=zs[:], in_=z1[:], axis=AX.X, op=OP.add)
        zr = work.tile([P, 1], F32)
        nc.vector.reciprocal(out=zr[:], in_=zs[:])
        cont_big = big_ps.tile([P, C, K], F32, tag="big")
        cont_ps = cont_big[:, 0:3, :].rearrange("p a b -> p (a b)")
        for r in range(2):
            for n0, n1 in ((0, 512), (512, D)):
                nc.tensor.matmul(
                    cont_ps[:, n0:n1],
                    lhsT=e1t_sb[:, r * 128:(r + 1) * 128],
                    rhs=m_sb[r][:, n0:n1],
                    start=(r == 0), stop=(r == 1),
                )
        cont_sb = work.tile([P, D], F32)
        nc.vector.tensor_scalar_mul(cont_sb[:], cont_ps[:], zr[:])
        nc.sync.dma_start(content[:], cont_sb[:])


_NC_CACHE = None


def _get_nc():
    global _NC_CACHE
    if _NC_CACHE is None:
        _NC_CACHE = build_kernel()
    return _NC_CACHE


def _consts():
    ident = np.eye(128, dtype=np.float32)
    emq = np.zeros((C, C, 128), np.float32)
    emk = np.zeros((C, C, K), np.float32)
    for c in range(C):
        emq[c, c, :] = 1.0
        emk[c, c, :] = 1.0
    iota = np.arange(128, dtype=np.float32).reshape(128, 1)
    return (ident, np.ascontiguousarray(emq.reshape(C, C * 128)),
            np.ascontiguousarray(emk.reshape(C, C * K)), iota)


def make_in_maps(q, k, m, attention_mask, emb, W1, b1, W2, b2, length_mask):
    q = np.ascontiguousarray(np.asarray(q, np.float32))
    k = np.ascontiguousarray(np.asarray(k, np.float32))
    m = np.ascontiguousarray(np.asarray(m, np.float32))
    am = np.ascontiguousarray(np.asarray(attention_mask, np.float32))
    emb = np.ascontiguousarray(np.asarray(emb, np.float32))
    lm = np.asarray(length_mask).astype(np.float16)
    w12 = np.ascontiguousarray(
        np.concatenate([np.asarray(W1, np.float32), np.asarray(W2, np.float32)], 0)
    )
    b12 = np.ascontiguousarray(
        np.concatenate(
            [np.asarray(b1, np.float32), np.asarray(b2, np.float32)]
        ).reshape(1, C)
    )
    ident, emq, emk, iota = _consts()
    in_maps = []
    for i in range(NCORES):
        b, qh = divmod(i, 2)
        sl = slice(qh * P, (qh + 1) * P)
        lmf = np.ascontiguousarray(lm[b, sl].T.reshape(1, P * K))  # k-major
        in_maps.append({
            "qs": q[b, sl],
            "ks": k[b],
            "ms": m[b],
            "emb": emb,
            "w12": w12,
            "b12": b12,
            "am": am[b, sl],
            "lmf": lmf,
            "ident": ident,
            "emq": emq,
            "emk": emk,
            "iota": iota,
        })
    return in_maps


def assemble_outputs(results):
    content = np.empty((B, Q, D), np.float32)
    scores = np.empty((B, Q, K, 5), np.float32)
    for i in range(NCORES):
        b, qh = divmod(i, 2)
        sl = slice(qh * P, (qh + 1) * P)
        content[b, sl] = results[i]["content"]
        scores[b, sl] = results[i]["scores"].reshape(P, K, 5)
    return content, scores


def kernel(**inputs):
    from concourse.bass_utils import run_bass_kernel_spmd

    nc = _get_nc()
    in_maps = make_in_maps(**inputs)
    res = run_bass_kernel_spmd(nc, in_maps, list(range(NCORES)))
    return assemble_outputs(res.results)
